# revision 5
# baseline (speedup 1.0000x reference)
"""Trainium2 Bass kernel for nn_NeuralDevice (segment_reduce), v3.

Architecture (per reference):
  two "eyes": h = relu(x @ Wr + br)            [N=1M, 64] -> [N, 128]
              segment-mean over idx (B=65536)  -> [B, 128]
              e = relu(mean @ Wc + bc)         -> [B, 128]
  brain:      z = [e0, e1]; out = relu(z@Wb1+bb1) @ Wb2 + bb2 -> [B, 128]

Distribution: shuffle-by-key, 8 cores x 8192 segments.  Host sorts each
core's nodes by segment, prescales each row by 1/max(cnt,1) (so segment
SUM == segment MEAN, no count column, no reciprocal fin), and packs the
64-feature rows of chunk PAIRS into the 128 SBUF partitions so mm1 runs
as two row-group-tiled matmuls per LDWEIGHTS window (K=64 each).

On-device per 128-row chunk:
  mm1 (pair-packed):  h_psum[128 rows, 128] = x_chunk^T.T @ Wr
  relu (ACT/DVE alternating): h bf16 -> SBUF
  mm2: meanT_psum[128 feats, 64 segs] += h^T @ sel   (sel one-hot row->seg,
       either DVE-generated from iota==segid or DMA'd (bf16/fp8))
Windows are 64 segments; 8 windows (512 segs) share one PSUM bank; when a
group completes: copy psum->sbuf bf16 (this IS meanT, thanks to the
prescale), eT = relu(Wc^T @ meanT + bc) -> persistent [128, 8192] per eye.
Brain MLP feat-major over 512-segment tiles as before.
"""

import numpy as np
import ml_dtypes

from concourse import bass, mybir
import concourse.bacc as bacc
import concourse.tile as tile
from concourse.bass_utils import run_bass_kernel_spmd

BF16 = ml_dtypes.bfloat16
FP8 = ml_dtypes.float8_e4m3fn

# problem sizes (hardcoded per spec)
B_FULL = 65536
N_FULL = 1048576
IN_NF = 64
R_OUT = 128
C_OUT = 128
BRAIN_H = 256
BRAIN_OUT = 128

CORES = 8
SEGS = B_FULL // CORES      # 8192 segments per core
WIN = 64                    # segments per accumulation window
WGRP = 8                    # windows per PSUM group (512 segs)
HB = 8                      # chunks per h-psum batch / relu batch
XCOLS = 4096                # packed-x columns per DMA tile (32 pairs)
SELCH = 96                  # chunks per sel DMA tile (dma modes)

# 'dve'  : selector generated on DVE via iota==segid compare
# 'bf16' : selector DMA'd as bf16 one-hot
# 'fp8'  : selector DMA'd as fp8e4 one-hot (mixed-dtype matmul)
SEL_MODE = "dve"
# fraction of relu batches on ACT (rest on DVE), as pattern out of 8
ACT_RELU_OF8 = 4

f32 = mybir.dt.float32
bf16 = mybir.dt.bfloat16
fp8e4 = mybir.dt.float8e4
RELU = mybir.ActivationFunctionType.Relu


# ----------------------------------------------------------------- planning

def _plan_eye(idx):
    """Per-eye shared window schedule + per-core sorted node placement."""
    n_win = SEGS // WIN
    owner = idx // SEGS
    per_c = {}
    runs = np.zeros((CORES, n_win), np.int64)
    for c in range(CORES):
        nodes = np.flatnonzero(owner == c)
        srel = idx[nodes] - c * SEGS
        order = np.argsort(srel, kind="stable")
        nodes = nodes[order]
        srel = srel[order]
        per_c[c] = (nodes, srel)
        runs[c] = np.bincount(srel // WIN, minlength=n_win)
    win_sizes = ((runs.max(axis=0) + 127) // 128) * 128
    win_sizes = np.maximum(win_sizes, 128)
    if (int(win_sizes.sum()) // 128) % 2:
        win_sizes[-1] += 128
    return win_sizes.tolist(), per_c


def _eye_sched(win_sizes):
    woc = []
    for w, sz in enumerate(win_sizes):
        woc.extend([w] * (sz // 128))
    first = {}
    last = {}
    for c, w in enumerate(woc):
        first.setdefault(w, c)
        last[w] = c
    return woc, first, last


# ------------------------------------------------------------ program build

_NC_CACHE = {}


def _build_nc(key):
    if key in _NC_CACHE:
        return _NC_CACHE[key]
    (ws0, ws1, sel_mode, has_br) = key
    win_sizes = [list(ws0), list(ws1)]
    scheds = [_eye_sched(win_sizes[0]), _eye_sched(win_sizes[1])]
    nchunks = [len(scheds[0][0]), len(scheds[1][0])]
    n_win = SEGS // WIN

    nc = bacc.Bacc("TRN2", target_bir_lowering=False, debug=False)

    xp_d = [nc.dram_tensor(f"x{e}p", [128, nchunks[e] * 64], bf16,
                           kind="ExternalInput") for e in range(2)]
    wr2_d = [nc.dram_tensor(f"wr{e}", [128, R_OUT], bf16, kind="ExternalInput")
             for e in range(2)]
    wc_d = [nc.dram_tensor(f"wc{e}", [R_OUT, C_OUT], bf16, kind="ExternalInput")
            for e in range(2)]
    bc_d = [nc.dram_tensor(f"bc{e}", [C_OUT, 1], f32, kind="ExternalInput")
            for e in range(2)]
    if sel_mode == "dve":
        seg_d = [nc.dram_tensor(f"seg{e}", [128, nchunks[e]], f32,
                                kind="ExternalInput") for e in range(2)]
        iota_d = nc.dram_tensor("iota", [128, WIN], bf16, kind="ExternalInput")
        sel_d = None
        seldt = bf16
    else:
        seldt = bf16 if sel_mode == "bf16" else fp8e4
        sel_d = [nc.dram_tensor(f"sel{e}", [128, nchunks[e] * WIN], seldt,
                                kind="ExternalInput") for e in range(2)]
        seg_d = None
    if has_br:
        invr_d = [nc.dram_tensor(f"invr{e}", [1, nchunks[e] * 128], f32,
                                 kind="ExternalInput") for e in range(2)]
        br_d = [nc.dram_tensor(f"br{e}", [1, R_OUT], bf16, kind="ExternalInput")
                for e in range(2)]
    wb1lo_d = nc.dram_tensor("wb1lo", [128, BRAIN_H], bf16, kind="ExternalInput")
    wb1hi_d = nc.dram_tensor("wb1hi", [128, BRAIN_H], bf16, kind="ExternalInput")
    bb1a_d = nc.dram_tensor("bb1a", [128, 1], f32, kind="ExternalInput")
    bb1b_d = nc.dram_tensor("bb1b", [128, 1], f32, kind="ExternalInput")
    wb2lo_d = nc.dram_tensor("wb2lo", [128, BRAIN_OUT], bf16, kind="ExternalInput")
    wb2hi_d = nc.dram_tensor("wb2hi", [128, BRAIN_OUT], bf16, kind="ExternalInput")
    bb2_d = nc.dram_tensor("bb2", [BRAIN_OUT, 1], f32, kind="ExternalInput")
    outT_d = nc.dram_tensor("outT", [128, SEGS], f32, kind="ExternalOutput")

    with tile.TileContext(nc) as tc:
        with tc.tile_pool(name="consts", bufs=1) as cp:
            wr2_t = [cp.tile([128, R_OUT], bf16, tag=f"wr{e}", name=f"wr{e}t")
                     for e in range(2)]
            wc_t = [cp.tile([R_OUT, C_OUT], bf16, tag=f"wc{e}", name=f"wc{e}t")
                    for e in range(2)]
            bc_t = [cp.tile([C_OUT, 1], f32, tag=f"bc{e}", name=f"bc{e}t")
                    for e in range(2)]
            wb1lo_t = cp.tile([128, BRAIN_H], bf16, tag="wb1lo")
            wb1hi_t = cp.tile([128, BRAIN_H], bf16, tag="wb1hi")
            bb1a_t = cp.tile([128, 1], f32, tag="bb1a")
            bb1b_t = cp.tile([128, 1], f32, tag="bb1b")
            wb2lo_t = cp.tile([128, BRAIN_OUT], bf16, tag="wb2lo")
            wb2hi_t = cp.tile([128, BRAIN_OUT], bf16, tag="wb2hi")
            bb2_t = cp.tile([BRAIN_OUT, 1], f32, tag="bb2")
            for e in range(2):
                nc.sync.dma_start(out=wr2_t[e][:], in_=wr2_d[e][:])
                nc.sync.dma_start(out=wc_t[e][:], in_=wc_d[e][:])
                nc.sync.dma_start(out=bc_t[e][:], in_=bc_d[e][:])
            nc.sync.dma_start(out=wb1lo_t[:], in_=wb1lo_d[:])
            nc.sync.dma_start(out=wb1hi_t[:], in_=wb1hi_d[:])
            nc.sync.dma_start(out=bb1a_t[:], in_=bb1a_d[:])
            nc.sync.dma_start(out=bb1b_t[:], in_=bb1b_d[:])
            nc.sync.dma_start(out=wb2lo_t[:], in_=wb2lo_d[:])
            nc.sync.dma_start(out=wb2hi_t[:], in_=wb2hi_d[:])
            nc.sync.dma_start(out=bb2_t[:], in_=bb2_d[:])
            if sel_mode == "dve":
                iota_t = cp.tile([128, WIN], bf16, tag="iota")
                nc.sync.dma_start(out=iota_t[:], in_=iota_d[:])
                seg_t = [cp.tile([128, nchunks[e]], f32, tag=f"seg{e}",
                                 name=f"seg{e}t") for e in range(2)]
                for e in range(2):
                    nc.sync.dma_start(out=seg_t[e][:], in_=seg_d[e][:])
            if has_br:
                br_t = [cp.tile([1, R_OUT], bf16, tag=f"br{e}", name=f"br{e}t")
                        for e in range(2)]
                for e in range(2):
                    nc.sync.dma_start(out=br_t[e][:], in_=br_d[e][:])

            eT_t = [cp.tile([128, SEGS], bf16, tag=f"eT{e}", name=f"eT{e}t")
                    for e in range(2)]

            # ------------------------------------------------ main phase
            with (
                tc.tile_pool(name="xch", bufs=3) as xpool,
                tc.tile_pool(name="selp", bufs=4) as selp,
                tc.tile_pool(name="hs", bufs=4) as hpool,
                tc.tile_pool(name="fins", bufs=2) as fs,
                tc.tile_pool(name="invp", bufs=2) as invp,
                tc.tile_pool(name="hps", bufs=2, space="PSUM") as hpp,
                tc.tile_pool(name="winp", bufs=2, space="PSUM") as wpp,
                tc.tile_pool(name="wcp", bufs=2, space="PSUM") as wcp,
            ):
                relu_ct = 0
                for e in range(2):
                    woc, wfirst, wlast = scheds[e]
                    nch = nchunks[e]
                    xt = None
                    invt = None
                    selt = None
                    wacc = None
                    pending = None

                    def emit_mm2(c0, n, hsb, state):
                        nonlocal wacc, selt
                        for j in range(n):
                            c = c0 + j
                            w = woc[c]
                            g = w // WGRP
                            if sel_mode == "dve":
                                sel = selp.tile([128, WIN], bf16, tag="sel",
                                                name=f"sel{e}_{c}")
                                nc.vector.tensor_scalar(
                                    out=sel[:], in0=iota_t[:],
                                    scalar1=seg_t[e][:, c:c + 1],
                                    scalar2=None,
                                    op0=mybir.AluOpType.is_equal)
                                rhs = sel[:]
                            else:
                                if c % SELCH == 0:
                                    scnt = min(SELCH, nch - c)
                                    selt = selp.tile([128, SELCH * WIN], seldt,
                                                     tag="selt",
                                                     name=f"selt{e}_{c}")
                                    nc.sync.dma_start(
                                        out=selt[:, : scnt * WIN],
                                        in_=sel_d[e][:, c * WIN:
                                                     (c + scnt) * WIN])
                                off = (c % SELCH) * WIN
                                rhs = selt[:, off:off + WIN]
                            if c == wfirst[g * WGRP]:
                                wacc = wpp.tile([128, WGRP * WIN], f32,
                                                tag="wacc", name=f"wa{e}_{g}")
                            ws = (w % WGRP) * WIN
                            slot = (j >> 1) + (j & 1) * (HB // 2)
                            nc.tensor.matmul(
                                out=wacc[:, ws:ws + WIN],
                                lhsT=hsb[:, slot * 128:(slot + 1) * 128],
                                rhs=rhs,
                                start=(c == wfirst[w]),
                                stop=(c == wlast[w]),
                            )
                            if c == wlast[w] and w % WGRP == WGRP - 1:
                                meanT = fs.tile([128, WGRP * WIN], bf16,
                                                tag="meanT", name=f"mt{e}_{g}")
                                nc.scalar.copy(out=meanT[:], in_=wacc[:])
                                pse = wcp.tile([128, WGRP * WIN], f32,
                                               tag="pse", name=f"pse{e}_{g}")
                                nc.tensor.matmul(out=pse[:], lhsT=wc_t[e][:],
                                                 rhs=meanT[:],
                                                 start=True, stop=True)
                                nc.scalar.activation(
                                    out=eT_t[e][:, g * 512:(g + 1) * 512],
                                    in_=pse[:], func=RELU,
                                    bias=bc_t[e][:, 0:1])

                    for c0 in range(0, nch, HB):
                        n = min(HB, nch - c0)
                        hps = hpp.tile([128, HB * 128], f32, tag="hps",
                                       name=f"hps{e}_{c0}")
                        hsb = hpool.tile([128, HB * 128], bf16, tag="hsb",
                                         name=f"hsb{e}_{c0}")
                        for t in range(n // 2):
                            pair = c0 // 2 + t
                            if pair % (XCOLS // 128) == 0:
                                pbase = pair * 128
                                pcsz = min(XCOLS, nch * 64 - pbase)
                                xt = xpool.tile([128, XCOLS], bf16, tag="xch",
                                                name=f"xch{e}_{pair}")
                                nc.sync.dma_start(
                                    out=xt[:, :pcsz],
                                    in_=xp_d[e][:, pbase:pbase + pcsz])
                                if has_br:
                                    ibase = pair * 256
                                    icsz = min(2 * XCOLS, nch * 128 - ibase)
                                    invt = invp.tile([1, 2 * XCOLS], f32,
                                                     tag="invr",
                                                     name=f"invr{e}_{pair}")
                                    nc.sync.dma_start(
                                        out=invt[:, :icsz],
                                        in_=invr_d[e][:, ibase:ibase + icsz])
                            col = (pair % (XCOLS // 128)) * 128
                            for half in range(2):
                                # row-tiled pair: A -> bank0 slot t,
                                # B -> bank1 slot HB//2+t (concurrent row
                                # tiles must write different PSUM banks)
                                slot = t + half * (HB // 2)
                                hs = slice(slot * 128, (slot + 1) * 128)
                                pb = half * 64
                                nc.tensor.matmul(
                                    out=hps[:, hs],
                                    lhsT=xt[pb:pb + 64, col:col + 128],
                                    rhs=wr2_t[e][pb:pb + 64, :],
                                    start=True, stop=not has_br,
                                )
                                if has_br:
                                    ic = (pair % (XCOLS // 128)) * 256 \
                                        + half * 128
                                    nc.tensor.matmul(
                                        out=hps[:, hs],
                                        lhsT=invt[0:1, ic:ic + 128],
                                        rhs=br_t[e][0:1, :],
                                        start=False, stop=True,
                                    )
                        for bank in range(2):
                            hh = bank * (HB // 2) * 128
                            hsz = (n // 2) * 128
                            if relu_ct % 8 < ACT_RELU_OF8:
                                nc.scalar.activation(
                                    out=hsb[:, hh:hh + hsz],
                                    in_=hps[:, hh:hh + hsz], func=RELU)
                            else:
                                nc.vector.tensor_scalar_max(
                                    hsb[:, hh:hh + hsz],
                                    hps[:, hh:hh + hsz], 0.0)
                            relu_ct += 1
                        if pending is not None:
                            emit_mm2(*pending, None)
                        pending = (c0, n, hsb)
                    emit_mm2(*pending, None)
                    pending = None

            # ------------------------------------------------ brain phase
            with (
                tc.tile_pool(name="bs", bufs=3) as bs,
                tc.tile_pool(name="bph", bufs=2, space="PSUM") as bph,
                tc.tile_pool(name="bpy", bufs=2, space="PSUM") as bpy,
            ):
                for t in range(SEGS // 512):
                    r0 = t * 512
                    e0s = eT_t[0][:, r0:r0 + 512]
                    e1s = eT_t[1][:, r0:r0 + 512]
                    psh_a = bph.tile([128, 512], f32, tag="bph", name=f"pha{t}")
                    nc.tensor.matmul(out=psh_a[:], lhsT=wb1lo_t[:, 0:128],
                                     rhs=e0s, start=True, stop=False)
                    nc.tensor.matmul(out=psh_a[:], lhsT=wb1hi_t[:, 0:128],
                                     rhs=e1s, start=False, stop=True)
                    hTa = bs.tile([128, 512], bf16, tag="hTa", name=f"hTa{t}")
                    nc.scalar.activation(out=hTa[:], in_=psh_a[:], func=RELU,
                                         bias=bb1a_t[:, 0:1])
                    psh_b = bph.tile([128, 512], f32, tag="bph", name=f"phb{t}")
                    nc.tensor.matmul(out=psh_b[:], lhsT=wb1lo_t[:, 128:256],
                                     rhs=e0s, start=True, stop=False)
                    nc.tensor.matmul(out=psh_b[:], lhsT=wb1hi_t[:, 128:256],
                                     rhs=e1s, start=False, stop=True)
                    hTb = bs.tile([128, 512], bf16, tag="hTb", name=f"hTb{t}")
                    nc.scalar.activation(out=hTb[:], in_=psh_b[:], func=RELU,
                                         bias=bb1b_t[:, 0:1])
                    psy = bpy.tile([128, 512], f32, tag="bpy", name=f"py{t}")
                    nc.tensor.matmul(out=psy[:], lhsT=wb2lo_t[:], rhs=hTa[:],
                                     start=True, stop=False)
                    nc.tensor.matmul(out=psy[:], lhsT=wb2hi_t[:], rhs=hTb[:],
                                     start=False, stop=True)
                    ys = bs.tile([128, 512], f32, tag="ys", name=f"ys{t}")
                    nc.vector.tensor_scalar_add(ys[:], psy[:], bb2_t[:, 0:1])
                    nc.sync.dma_start(out=outT_d[:, r0:r0 + 512], in_=ys[:])

    nc.compile()
    _NC_CACHE[key] = nc
    return nc


# ------------------------------------------------------------------ driver

def _prepare(inputs):
    x = [np.asarray(inputs["x0"], np.float32),
         np.asarray(inputs["x1"], np.float32)]
    idx = [np.asarray(inputs["idx0"]).astype(np.int64),
           np.asarray(inputs["idx1"]).astype(np.int64)]
    br = [np.asarray(inputs["br0"], np.float32),
          np.asarray(inputs["br1"], np.float32)]
    has_br = bool(np.any(br[0]) or np.any(br[1]))

    plans = [_plan_eye(idx[0]), _plan_eye(idx[1])]
    win_sizes = [plans[0][0], plans[1][0]]
    win_base = [np.cumsum([0] + ws) for ws in win_sizes]
    totals = [int(sum(ws)) for ws in win_sizes]
    n_win = SEGS // WIN

    invc = [
        (1.0 / np.maximum(
            np.bincount(idx[e], minlength=B_FULL), 1)).astype(np.float32)
        for e in range(2)
    ]

    shared = {}
    for e in range(2):
        wr = np.asarray(inputs[f"Wr{e}"], np.float32)
        shared[f"wr{e}"] = np.concatenate([wr, wr], axis=0).astype(BF16)
        shared[f"wc{e}"] = np.asarray(inputs[f"Wc{e}"], np.float32).astype(BF16)
        shared[f"bc{e}"] = np.asarray(
            inputs[f"bc{e}"], np.float32).reshape(-1, 1)
        if has_br:
            shared[f"br{e}"] = br[e].astype(BF16).reshape(1, -1)
    wb1 = np.asarray(inputs["Wb1"], np.float32)
    bb1 = np.asarray(inputs["bb1"], np.float32)
    wb2 = np.asarray(inputs["Wb2"], np.float32)
    bb2 = np.asarray(inputs["bb2"], np.float32)
    shared["wb1lo"] = wb1[0:128].astype(BF16)
    shared["wb1hi"] = wb1[128:256].astype(BF16)
    shared["bb1a"] = bb1[0:128].reshape(-1, 1)
    shared["bb1b"] = bb1[128:256].reshape(-1, 1)
    shared["wb2lo"] = wb2[0:128].astype(BF16)
    shared["wb2hi"] = wb2[128:256].astype(BF16)
    shared["bb2"] = bb2.reshape(-1, 1)
    if SEL_MODE == "dve":
        shared["iota"] = np.broadcast_to(
            np.arange(WIN, dtype=np.float32), (128, WIN)).astype(BF16)

    in_maps = []
    for c in range(CORES):
        m = dict(shared)
        for e in range(2):
            nodes, srel = plans[e][1][c]
            total = totals[e]
            nchunks = total // 128
            # stream position: window-aligned with per-window padding
            wid = srel // WIN
            wstart = np.searchsorted(wid, np.arange(n_win))
            pos = np.empty(len(nodes), np.int64)
            for w in range(n_win):
                lo = wstart[w]
                hi = wstart[w + 1] if w + 1 < n_win else len(nodes)
                pos[lo:hi] = win_base[e][w] + np.arange(hi - lo)
            arr = np.zeros((total, IN_NF), np.float32)
            arr[pos] = x[e][nodes] * invc[e][idx[e][nodes]][:, None]
            a3 = arr.reshape(nchunks, 128, IN_NF).astype(BF16)
            xp = np.concatenate([a3[0::2], a3[1::2]], axis=2)  # [P, 128, 128]
            m[f"x{e}p"] = np.ascontiguousarray(
                xp.transpose(2, 0, 1).reshape(128, total // 2))
            segv = np.full(total, -1.0, np.float32)
            segv[pos] = (srel % WIN).astype(np.float32)
            if SEL_MODE == "dve":
                m[f"seg{e}"] = np.ascontiguousarray(
                    segv.reshape(nchunks, 128).T)
            else:
                sdt = BF16 if SEL_MODE == "bf16" else FP8
                sel = (segv[:, None] ==
                       np.arange(WIN, dtype=np.float32)[None, :])
                m[f"sel{e}"] = np.ascontiguousarray(
                    sel.reshape(nchunks, 128, WIN).transpose(1, 0, 2)
                    .reshape(128, nchunks * WIN)).astype(sdt)
            if has_br:
                iv = np.zeros(total, np.float32)
                iv[pos] = invc[e][idx[e][nodes]]
                m[f"invr{e}"] = iv.reshape(1, total)
        in_maps.append(m)
    key = (tuple(win_sizes[0]), tuple(win_sizes[1]), SEL_MODE, has_br)
    return key, in_maps


def _axon_reset():
    try:
        import ctypes

        lib = ctypes.CDLL("/opt/axon/libaxon_pjrt.so")
        lib.axon_reset.restype = ctypes.c_int
        lib.axon_reset()
    except Exception:
        pass


def _run(inputs, trace=False, trace_kwargs=None):
    key, in_maps = _prepare(inputs)
    nc = _build_nc(key)
    try:
        res = run_bass_kernel_spmd(nc, in_maps, list(range(CORES)),
                                   trace=trace, **(trace_kwargs or {}))
    except Exception as e:
        if "UNRECOVERABLE" not in str(e) and "UNAVAILABLE" not in str(e):
            raise
        _axon_reset()
        res = run_bass_kernel_spmd(nc, in_maps, list(range(CORES)),
                                   trace=trace, **(trace_kwargs or {}))
    out = np.concatenate([res.results[c]["outT"].T for c in range(CORES)],
                         axis=0)
    return out.astype(np.float32), res


def kernel(**inputs):
    return _run(inputs)[0]


# revision 6
# speedup vs baseline: 2.2849x; 2.2849x over previous
"""Trainium2 Bass kernel for nn_NeuralDevice (segment_reduce), v3.

Architecture (per reference):
  two "eyes": h = relu(x @ Wr + br)            [N=1M, 64] -> [N, 128]
              segment-mean over idx (B=65536)  -> [B, 128]
              e = relu(mean @ Wc + bc)         -> [B, 128]
  brain:      z = [e0, e1]; out = relu(z@Wb1+bb1) @ Wb2 + bb2 -> [B, 128]

Distribution: shuffle-by-key, 8 cores x 8192 segments.  Host sorts each
core's nodes by segment, prescales each row by 1/max(cnt,1) (so segment
SUM == segment MEAN, no count column, no reciprocal fin), and packs the
64-feature rows of chunk PAIRS into the 128 SBUF partitions so mm1 runs
as two row-group-tiled matmuls per LDWEIGHTS window (K=64 each).

On-device per 128-row chunk:
  mm1 (pair-packed):  h_psum[128 rows, 128] = x_chunk^T.T @ Wr
  relu (ACT/DVE alternating): h bf16 -> SBUF
  mm2: meanT_psum[128 feats, 64 segs] += h^T @ sel   (sel one-hot row->seg,
       either DVE-generated from iota==segid or DMA'd (bf16/fp8))
Windows are 64 segments; 8 windows (512 segs) share one PSUM bank; when a
group completes: copy psum->sbuf bf16 (this IS meanT, thanks to the
prescale), eT = relu(Wc^T @ meanT + bc) -> persistent [128, 8192] per eye.
Brain MLP feat-major over 512-segment tiles as before.
"""

import numpy as np
import ml_dtypes

from concourse import bass, mybir
import concourse.bacc as bacc
import concourse.tile as tile
from concourse.bass_utils import run_bass_kernel_spmd

BF16 = ml_dtypes.bfloat16
FP8 = ml_dtypes.float8_e4m3fn

# problem sizes (hardcoded per spec)
B_FULL = 65536
N_FULL = 1048576
IN_NF = 64
R_OUT = 128
C_OUT = 128
BRAIN_H = 256
BRAIN_OUT = 128

CORES = 8
SEGS = B_FULL // CORES      # 8192 segments per core
WIN = 64                    # segments per accumulation window
WGRP = 8                    # windows per PSUM group (512 segs)
HB = 8                      # chunks per h-psum batch / relu batch
XCOLS = 4096                # packed-x columns per DMA tile (32 pairs)
SELCH = 96                  # chunks per sel DMA tile (dma modes)

# 'dve'  : selector generated on DVE via iota==segid compare
# 'bf16' : selector DMA'd as bf16 one-hot
# 'fp8'  : selector DMA'd as fp8e4 one-hot (mixed-dtype matmul)
SEL_MODE = "fp8"
# fraction of relu batches on ACT (rest on DVE), as pattern out of 8
ACT_RELU_OF8 = 4

f32 = mybir.dt.float32
bf16 = mybir.dt.bfloat16
fp8e4 = mybir.dt.float8e4
RELU = mybir.ActivationFunctionType.Relu


# ----------------------------------------------------------------- planning

def _plan_eye(idx):
    """Per-eye shared window schedule + per-core sorted node placement."""
    n_win = SEGS // WIN
    owner = idx // SEGS
    per_c = {}
    runs = np.zeros((CORES, n_win), np.int64)
    for c in range(CORES):
        nodes = np.flatnonzero(owner == c)
        srel = idx[nodes] - c * SEGS
        order = np.argsort(srel, kind="stable")
        nodes = nodes[order]
        srel = srel[order]
        per_c[c] = (nodes, srel)
        runs[c] = np.bincount(srel // WIN, minlength=n_win)
    win_sizes = ((runs.max(axis=0) + 127) // 128) * 128
    win_sizes = np.maximum(win_sizes, 128)
    if (int(win_sizes.sum()) // 128) % 2:
        win_sizes[-1] += 128
    return win_sizes.tolist(), per_c


def _eye_sched(win_sizes):
    woc = []
    for w, sz in enumerate(win_sizes):
        woc.extend([w] * (sz // 128))
    first = {}
    last = {}
    for c, w in enumerate(woc):
        first.setdefault(w, c)
        last[w] = c
    return woc, first, last


# ------------------------------------------------------------ program build

_NC_CACHE = {}


def _build_nc(key):
    if key in _NC_CACHE:
        return _NC_CACHE[key]
    (ws0, ws1, sel_mode, has_br) = key
    win_sizes = [list(ws0), list(ws1)]
    scheds = [_eye_sched(win_sizes[0]), _eye_sched(win_sizes[1])]
    nchunks = [len(scheds[0][0]), len(scheds[1][0])]
    n_win = SEGS // WIN

    nc = bacc.Bacc("TRN2", target_bir_lowering=False, debug=False)

    xp_d = [nc.dram_tensor(f"x{e}p", [128, nchunks[e] * 64], bf16,
                           kind="ExternalInput") for e in range(2)]
    wr2_d = [nc.dram_tensor(f"wr{e}", [128, R_OUT], bf16, kind="ExternalInput")
             for e in range(2)]
    wc_d = [nc.dram_tensor(f"wc{e}", [R_OUT, C_OUT], bf16, kind="ExternalInput")
            for e in range(2)]
    bc_d = [nc.dram_tensor(f"bc{e}", [C_OUT, 1], f32, kind="ExternalInput")
            for e in range(2)]
    if sel_mode == "dve":
        seg_d = [nc.dram_tensor(f"seg{e}", [128, nchunks[e]], f32,
                                kind="ExternalInput") for e in range(2)]
        iota_d = nc.dram_tensor("iota", [128, WIN], bf16, kind="ExternalInput")
        sel_d = None
        seldt = bf16
    else:
        seldt = bf16 if sel_mode == "bf16" else fp8e4
        sel_d = [nc.dram_tensor(f"sel{e}", [128, nchunks[e] * WIN], seldt,
                                kind="ExternalInput") for e in range(2)]
        seg_d = None
    if has_br:
        invr_d = [nc.dram_tensor(f"invr{e}", [1, nchunks[e] * 128], f32,
                                 kind="ExternalInput") for e in range(2)]
        br_d = [nc.dram_tensor(f"br{e}", [1, R_OUT], bf16, kind="ExternalInput")
                for e in range(2)]
    wb1lo_d = nc.dram_tensor("wb1lo", [128, BRAIN_H], bf16, kind="ExternalInput")
    wb1hi_d = nc.dram_tensor("wb1hi", [128, BRAIN_H], bf16, kind="ExternalInput")
    bb1a_d = nc.dram_tensor("bb1a", [128, 1], f32, kind="ExternalInput")
    bb1b_d = nc.dram_tensor("bb1b", [128, 1], f32, kind="ExternalInput")
    wb2lo_d = nc.dram_tensor("wb2lo", [128, BRAIN_OUT], bf16, kind="ExternalInput")
    wb2hi_d = nc.dram_tensor("wb2hi", [128, BRAIN_OUT], bf16, kind="ExternalInput")
    bb2_d = nc.dram_tensor("bb2", [BRAIN_OUT, 1], f32, kind="ExternalInput")
    outT_d = nc.dram_tensor("outT", [128, SEGS], f32, kind="ExternalOutput")

    with tile.TileContext(nc) as tc:
        with tc.tile_pool(name="consts", bufs=1) as cp:
            wr2_t = [cp.tile([128, R_OUT], bf16, tag=f"wr{e}", name=f"wr{e}t")
                     for e in range(2)]
            wc_t = [cp.tile([R_OUT, C_OUT], bf16, tag=f"wc{e}", name=f"wc{e}t")
                    for e in range(2)]
            bc_t = [cp.tile([C_OUT, 1], f32, tag=f"bc{e}", name=f"bc{e}t")
                    for e in range(2)]
            wb1lo_t = cp.tile([128, BRAIN_H], bf16, tag="wb1lo")
            wb1hi_t = cp.tile([128, BRAIN_H], bf16, tag="wb1hi")
            bb1a_t = cp.tile([128, 1], f32, tag="bb1a")
            bb1b_t = cp.tile([128, 1], f32, tag="bb1b")
            wb2lo_t = cp.tile([128, BRAIN_OUT], bf16, tag="wb2lo")
            wb2hi_t = cp.tile([128, BRAIN_OUT], bf16, tag="wb2hi")
            bb2_t = cp.tile([BRAIN_OUT, 1], f32, tag="bb2")
            for e in range(2):
                nc.sync.dma_start(out=wr2_t[e][:], in_=wr2_d[e][:])
                nc.sync.dma_start(out=wc_t[e][:], in_=wc_d[e][:])
                nc.sync.dma_start(out=bc_t[e][:], in_=bc_d[e][:])
            nc.sync.dma_start(out=wb1lo_t[:], in_=wb1lo_d[:])
            nc.sync.dma_start(out=wb1hi_t[:], in_=wb1hi_d[:])
            nc.sync.dma_start(out=bb1a_t[:], in_=bb1a_d[:])
            nc.sync.dma_start(out=bb1b_t[:], in_=bb1b_d[:])
            nc.sync.dma_start(out=wb2lo_t[:], in_=wb2lo_d[:])
            nc.sync.dma_start(out=wb2hi_t[:], in_=wb2hi_d[:])
            nc.sync.dma_start(out=bb2_t[:], in_=bb2_d[:])
            if sel_mode == "dve":
                iota_t = cp.tile([128, WIN], bf16, tag="iota")
                nc.sync.dma_start(out=iota_t[:], in_=iota_d[:])
                seg_t = [cp.tile([128, nchunks[e]], f32, tag=f"seg{e}",
                                 name=f"seg{e}t") for e in range(2)]
                for e in range(2):
                    nc.sync.dma_start(out=seg_t[e][:], in_=seg_d[e][:])
            if has_br:
                br_t = [cp.tile([1, R_OUT], bf16, tag=f"br{e}", name=f"br{e}t")
                        for e in range(2)]
                for e in range(2):
                    nc.sync.dma_start(out=br_t[e][:], in_=br_d[e][:])

            eT_t = [cp.tile([128, SEGS], bf16, tag=f"eT{e}", name=f"eT{e}t")
                    for e in range(2)]

            # ------------------------------------------------ main phase
            with (
                tc.tile_pool(name="xch", bufs=3) as xpool,
                tc.tile_pool(name="selp", bufs=4) as selp,
                tc.tile_pool(name="hs", bufs=4) as hpool,
                tc.tile_pool(name="fins", bufs=2) as fs,
                tc.tile_pool(name="invp", bufs=2) as invp,
                tc.tile_pool(name="hps", bufs=2, space="PSUM") as hpp,
                tc.tile_pool(name="winp", bufs=2, space="PSUM") as wpp,
                tc.tile_pool(name="wcp", bufs=2, space="PSUM") as wcp,
            ):
                relu_ct = 0
                for e in range(2):
                    woc, wfirst, wlast = scheds[e]
                    nch = nchunks[e]
                    xt = None
                    invt = None
                    selt = None
                    wacc = None
                    pending = None

                    def emit_mm2(c0, n, hsb, state):
                        nonlocal wacc, selt
                        for j in range(n):
                            c = c0 + j
                            w = woc[c]
                            g = w // WGRP
                            if sel_mode == "dve":
                                sel = selp.tile([128, WIN], bf16, tag="sel",
                                                name=f"sel{e}_{c}")
                                nc.vector.tensor_scalar(
                                    out=sel[:], in0=iota_t[:],
                                    scalar1=seg_t[e][:, c:c + 1],
                                    scalar2=None,
                                    op0=mybir.AluOpType.is_equal)
                                rhs = sel[:]
                            else:
                                if c % SELCH == 0:
                                    scnt = min(SELCH, nch - c)
                                    selt = selp.tile([128, SELCH * WIN], seldt,
                                                     tag="selt",
                                                     name=f"selt{e}_{c}")
                                    nc.sync.dma_start(
                                        out=selt[:, : scnt * WIN],
                                        in_=sel_d[e][:, c * WIN:
                                                     (c + scnt) * WIN])
                                off = (c % SELCH) * WIN
                                rhs = selt[:, off:off + WIN]
                            if c == wfirst[g * WGRP]:
                                wacc = wpp.tile([128, WGRP * WIN], f32,
                                                tag="wacc", name=f"wa{e}_{g}")
                            ws = (w % WGRP) * WIN
                            slot = (j >> 1) + (j & 1) * (HB // 2)
                            nc.tensor.matmul(
                                out=wacc[:, ws:ws + WIN],
                                lhsT=hsb[:, slot * 128:(slot + 1) * 128],
                                rhs=rhs,
                                start=(c == wfirst[w]),
                                stop=(c == wlast[w]),
                            )
                            if c == wlast[w] and w % WGRP == WGRP - 1:
                                meanT = fs.tile([128, WGRP * WIN], bf16,
                                                tag="meanT", name=f"mt{e}_{g}")
                                nc.vector.tensor_copy(meanT[:], wacc[:])
                                pse = wcp.tile([128, WGRP * WIN], f32,
                                               tag="pse", name=f"pse{e}_{g}")
                                nc.tensor.matmul(out=pse[:], lhsT=wc_t[e][:],
                                                 rhs=meanT[:],
                                                 start=True, stop=True)
                                nc.scalar.activation(
                                    out=eT_t[e][:, g * 512:(g + 1) * 512],
                                    in_=pse[:], func=RELU,
                                    bias=bc_t[e][:, 0:1])

                    for c0 in range(0, nch, HB):
                        n = min(HB, nch - c0)
                        hps = hpp.tile([128, HB * 128], f32, tag="hps",
                                       name=f"hps{e}_{c0}")
                        hsb = hpool.tile([128, HB * 128], bf16, tag="hsb",
                                         name=f"hsb{e}_{c0}")
                        for t in range(n // 2):
                            pair = c0 // 2 + t
                            if pair % (XCOLS // 128) == 0:
                                pbase = pair * 128
                                pcsz = min(XCOLS, nch * 64 - pbase)
                                xt = xpool.tile([128, XCOLS], bf16, tag="xch",
                                                name=f"xch{e}_{pair}")
                                nc.sync.dma_start(
                                    out=xt[:, :pcsz],
                                    in_=xp_d[e][:, pbase:pbase + pcsz])
                                if has_br:
                                    ibase = pair * 256
                                    icsz = min(2 * XCOLS, nch * 128 - ibase)
                                    invt = invp.tile([1, 2 * XCOLS], f32,
                                                     tag="invr",
                                                     name=f"invr{e}_{pair}")
                                    nc.sync.dma_start(
                                        out=invt[:, :icsz],
                                        in_=invr_d[e][:, ibase:ibase + icsz])
                            col = (pair % (XCOLS // 128)) * 128
                            for half in range(2):
                                # row-tiled pair: A -> bank0 slot t,
                                # B -> bank1 slot HB//2+t (concurrent row
                                # tiles must write different PSUM banks)
                                slot = t + half * (HB // 2)
                                hs = slice(slot * 128, (slot + 1) * 128)
                                pb = half * 64
                                nc.tensor.matmul(
                                    out=hps[:, hs],
                                    lhsT=xt[pb:pb + 64, col:col + 128],
                                    rhs=wr2_t[e][pb:pb + 64, :],
                                    start=True, stop=not has_br,
                                )
                                if has_br:
                                    ic = (pair % (XCOLS // 128)) * 256 \
                                        + half * 128
                                    nc.tensor.matmul(
                                        out=hps[:, hs],
                                        lhsT=invt[0:1, ic:ic + 128],
                                        rhs=br_t[e][0:1, :],
                                        start=False, stop=True,
                                    )
                        if n == HB:
                            spans = [(0, HB * 128)]
                        else:
                            spans = [(0, (n // 2) * 128),
                                     ((HB // 2) * 128,
                                      (HB // 2) * 128 + (n // 2) * 128)]
                        for hh, hsz in spans:
                            if relu_ct % 3 < 2:
                                nc.scalar.activation(
                                    out=hsb[:, hh:hh + hsz],
                                    in_=hps[:, hh:hh + hsz], func=RELU)
                            else:
                                nc.vector.tensor_scalar_max(
                                    hsb[:, hh:hh + hsz],
                                    hps[:, hh:hh + hsz], 0.0)
                            relu_ct += 1
                        if pending is not None:
                            emit_mm2(*pending, None)
                        pending = (c0, n, hsb)
                    emit_mm2(*pending, None)
                    pending = None

            # ------------------------------------------------ brain phase
            with (
                tc.tile_pool(name="bs", bufs=3) as bs,
                tc.tile_pool(name="bph", bufs=2, space="PSUM") as bph,
                tc.tile_pool(name="bpy", bufs=2, space="PSUM") as bpy,
            ):
                for t in range(SEGS // 512):
                    r0 = t * 512
                    e0s = eT_t[0][:, r0:r0 + 512]
                    e1s = eT_t[1][:, r0:r0 + 512]
                    psh_a = bph.tile([128, 512], f32, tag="bph", name=f"pha{t}")
                    nc.tensor.matmul(out=psh_a[:], lhsT=wb1lo_t[:, 0:128],
                                     rhs=e0s, start=True, stop=False)
                    nc.tensor.matmul(out=psh_a[:], lhsT=wb1hi_t[:, 0:128],
                                     rhs=e1s, start=False, stop=True)
                    hTa = bs.tile([128, 512], bf16, tag="hTa", name=f"hTa{t}")
                    nc.scalar.activation(out=hTa[:], in_=psh_a[:], func=RELU,
                                         bias=bb1a_t[:, 0:1])
                    psh_b = bph.tile([128, 512], f32, tag="bph", name=f"phb{t}")
                    nc.tensor.matmul(out=psh_b[:], lhsT=wb1lo_t[:, 128:256],
                                     rhs=e0s, start=True, stop=False)
                    nc.tensor.matmul(out=psh_b[:], lhsT=wb1hi_t[:, 128:256],
                                     rhs=e1s, start=False, stop=True)
                    hTb = bs.tile([128, 512], bf16, tag="hTb", name=f"hTb{t}")
                    nc.scalar.activation(out=hTb[:], in_=psh_b[:], func=RELU,
                                         bias=bb1b_t[:, 0:1])
                    psy = bpy.tile([128, 512], f32, tag="bpy", name=f"py{t}")
                    nc.tensor.matmul(out=psy[:], lhsT=wb2lo_t[:], rhs=hTa[:],
                                     start=True, stop=False)
                    nc.tensor.matmul(out=psy[:], lhsT=wb2hi_t[:], rhs=hTb[:],
                                     start=False, stop=True)
                    ys = bs.tile([128, 512], f32, tag="ys", name=f"ys{t}")
                    nc.vector.tensor_scalar_add(ys[:], psy[:], bb2_t[:, 0:1])
                    nc.sync.dma_start(out=outT_d[:, r0:r0 + 512], in_=ys[:])

    nc.compile()
    _NC_CACHE[key] = nc
    return nc


# ------------------------------------------------------------------ driver

def _prepare(inputs):
    x = [np.asarray(inputs["x0"], np.float32),
         np.asarray(inputs["x1"], np.float32)]
    idx = [np.asarray(inputs["idx0"]).astype(np.int64),
           np.asarray(inputs["idx1"]).astype(np.int64)]
    br = [np.asarray(inputs["br0"], np.float32),
          np.asarray(inputs["br1"], np.float32)]
    has_br = bool(np.any(br[0]) or np.any(br[1]))

    plans = [_plan_eye(idx[0]), _plan_eye(idx[1])]
    win_sizes = [plans[0][0], plans[1][0]]
    win_base = [np.cumsum([0] + ws) for ws in win_sizes]
    totals = [int(sum(ws)) for ws in win_sizes]
    n_win = SEGS // WIN

    invc = [
        (1.0 / np.maximum(
            np.bincount(idx[e], minlength=B_FULL), 1)).astype(np.float32)
        for e in range(2)
    ]

    shared = {}
    for e in range(2):
        wr = np.asarray(inputs[f"Wr{e}"], np.float32)
        shared[f"wr{e}"] = np.concatenate([wr, wr], axis=0).astype(BF16)
        shared[f"wc{e}"] = np.asarray(inputs[f"Wc{e}"], np.float32).astype(BF16)
        shared[f"bc{e}"] = np.asarray(
            inputs[f"bc{e}"], np.float32).reshape(-1, 1)
        if has_br:
            shared[f"br{e}"] = br[e].astype(BF16).reshape(1, -1)
    wb1 = np.asarray(inputs["Wb1"], np.float32)
    bb1 = np.asarray(inputs["bb1"], np.float32)
    wb2 = np.asarray(inputs["Wb2"], np.float32)
    bb2 = np.asarray(inputs["bb2"], np.float32)
    shared["wb1lo"] = wb1[0:128].astype(BF16)
    shared["wb1hi"] = wb1[128:256].astype(BF16)
    shared["bb1a"] = bb1[0:128].reshape(-1, 1)
    shared["bb1b"] = bb1[128:256].reshape(-1, 1)
    shared["wb2lo"] = wb2[0:128].astype(BF16)
    shared["wb2hi"] = wb2[128:256].astype(BF16)
    shared["bb2"] = bb2.reshape(-1, 1)
    if SEL_MODE == "dve":
        shared["iota"] = np.broadcast_to(
            np.arange(WIN, dtype=np.float32), (128, WIN)).astype(BF16)

    in_maps = []
    for c in range(CORES):
        m = dict(shared)
        for e in range(2):
            nodes, srel = plans[e][1][c]
            total = totals[e]
            nchunks = total // 128
            # stream position: window-aligned with per-window padding
            wid = srel // WIN
            wstart = np.searchsorted(wid, np.arange(n_win))
            pos = np.empty(len(nodes), np.int64)
            for w in range(n_win):
                lo = wstart[w]
                hi = wstart[w + 1] if w + 1 < n_win else len(nodes)
                pos[lo:hi] = win_base[e][w] + np.arange(hi - lo)
            arr = np.zeros((total, IN_NF), np.float32)
            arr[pos] = x[e][nodes] * invc[e][idx[e][nodes]][:, None]
            a3 = arr.reshape(nchunks, 128, IN_NF).astype(BF16)
            xp = np.concatenate([a3[0::2], a3[1::2]], axis=2)  # [P, 128, 128]
            m[f"x{e}p"] = np.ascontiguousarray(
                xp.transpose(2, 0, 1).reshape(128, total // 2))
            segv = np.full(total, -1.0, np.float32)
            segv[pos] = (srel % WIN).astype(np.float32)
            if SEL_MODE == "dve":
                m[f"seg{e}"] = np.ascontiguousarray(
                    segv.reshape(nchunks, 128).T)
            else:
                sdt = BF16 if SEL_MODE == "bf16" else FP8
                sel = (segv[:, None] ==
                       np.arange(WIN, dtype=np.float32)[None, :])
                m[f"sel{e}"] = np.ascontiguousarray(
                    sel.reshape(nchunks, 128, WIN).transpose(1, 0, 2)
                    .reshape(128, nchunks * WIN)).astype(sdt)
            if has_br:
                iv = np.zeros(total, np.float32)
                iv[pos] = invc[e][idx[e][nodes]]
                m[f"invr{e}"] = iv.reshape(1, total)
        in_maps.append(m)
    key = (tuple(win_sizes[0]), tuple(win_sizes[1]), SEL_MODE, has_br)
    return key, in_maps


def _axon_reset():
    try:
        import ctypes

        lib = ctypes.CDLL("/opt/axon/libaxon_pjrt.so")
        lib.axon_reset.restype = ctypes.c_int
        lib.axon_reset()
    except Exception:
        pass


def _run(inputs, trace=False, trace_kwargs=None):
    key, in_maps = _prepare(inputs)
    nc = _build_nc(key)
    try:
        res = run_bass_kernel_spmd(nc, in_maps, list(range(CORES)),
                                   trace=trace, **(trace_kwargs or {}))
    except Exception as e:
        if "UNRECOVERABLE" not in str(e) and "UNAVAILABLE" not in str(e):
            raise
        _axon_reset()
        res = run_bass_kernel_spmd(nc, in_maps, list(range(CORES)),
                                   trace=trace, **(trace_kwargs or {}))
    out = np.concatenate([res.results[c]["outT"].T for c in range(CORES)],
                         axis=0)
    return out.astype(np.float32), res


def kernel(**inputs):
    return _run(inputs)[0]


# revision 10
# speedup vs baseline: 2.3138x; 1.0127x over previous
"""Trainium2 Bass kernel for nn_NeuralDevice (segment_reduce), v3.

Architecture (per reference):
  two "eyes": h = relu(x @ Wr + br)            [N=1M, 64] -> [N, 128]
              segment-mean over idx (B=65536)  -> [B, 128]
              e = relu(mean @ Wc + bc)         -> [B, 128]
  brain:      z = [e0, e1]; out = relu(z@Wb1+bb1) @ Wb2 + bb2 -> [B, 128]

Distribution: shuffle-by-key, 8 cores x 8192 segments.  Host sorts each
core's nodes by segment, prescales each row by 1/max(cnt,1) (so segment
SUM == segment MEAN, no count column, no reciprocal fin), and packs the
64-feature rows of chunk PAIRS into the 128 SBUF partitions so mm1 runs
as two row-group-tiled matmuls per LDWEIGHTS window (K=64 each).

On-device per 128-row chunk:
  mm1 (pair-packed):  h_psum[128 rows, 128] = x_chunk^T.T @ Wr
  relu (ACT/DVE alternating): h bf16 -> SBUF
  mm2: meanT_psum[128 feats, 64 segs] += h^T @ sel   (sel one-hot row->seg,
       either DVE-generated from iota==segid or DMA'd (bf16/fp8))
Windows are 64 segments; 8 windows (512 segs) share one PSUM bank; when a
group completes: copy psum->sbuf bf16 (this IS meanT, thanks to the
prescale), eT = relu(Wc^T @ meanT + bc) -> persistent [128, 8192] per eye.
Brain MLP feat-major over 512-segment tiles as before.
"""

import numpy as np
import ml_dtypes

from concourse import bass, mybir
import concourse.bacc as bacc
import concourse.tile as tile
from concourse.bass_utils import run_bass_kernel_spmd

BF16 = ml_dtypes.bfloat16
FP8 = ml_dtypes.float8_e4m3fn

# problem sizes (hardcoded per spec)
B_FULL = 65536
N_FULL = 1048576
IN_NF = 64
R_OUT = 128
C_OUT = 128
BRAIN_H = 256
BRAIN_OUT = 128

CORES = 8
SEGS = B_FULL // CORES      # 8192 segments per core
WIN = 64                    # segments per accumulation window
WGRP = 8                    # windows per PSUM group (512 segs)
HB = 8                      # chunks per h-psum batch / relu batch
XCOLS = 4096                # packed-x columns per DMA tile (32 pairs)
SELCH = 96                  # chunks per sel DMA tile (dma modes)

# 'dve'  : selector generated on DVE via iota==segid compare
# 'bf16' : selector DMA'd as bf16 one-hot
# 'fp8'  : selector DMA'd as fp8e4 one-hot (mixed-dtype matmul)
SEL_MODE = "fp8"
# fraction of relu batches on ACT (rest on DVE), as pattern out of 8
ACT_RELU_OF8 = 4

f32 = mybir.dt.float32
bf16 = mybir.dt.bfloat16
fp8e4 = mybir.dt.float8e4
RELU = mybir.ActivationFunctionType.Relu


# ----------------------------------------------------------------- planning

def _plan_eye(idx):
    """Per-eye shared window schedule + per-core sorted node placement."""
    n_win = SEGS // WIN
    owner = idx // SEGS
    per_c = {}
    runs = np.zeros((CORES, n_win), np.int64)
    for c in range(CORES):
        nodes = np.flatnonzero(owner == c)
        srel = idx[nodes] - c * SEGS
        order = np.argsort(srel, kind="stable")
        nodes = nodes[order]
        srel = srel[order]
        per_c[c] = (nodes, srel)
        runs[c] = np.bincount(srel // WIN, minlength=n_win)
    win_sizes = ((runs.max(axis=0) + 127) // 128) * 128
    win_sizes = np.maximum(win_sizes, 128)
    if (int(win_sizes.sum()) // 128) % 2:
        win_sizes[-1] += 128
    return win_sizes.tolist(), per_c


def _eye_sched(win_sizes):
    woc = []
    for w, sz in enumerate(win_sizes):
        woc.extend([w] * (sz // 128))
    first = {}
    last = {}
    for c, w in enumerate(woc):
        first.setdefault(w, c)
        last[w] = c
    return woc, first, last


# ------------------------------------------------------------ program build

_NC_CACHE = {}


def _build_nc(key):
    if key in _NC_CACHE:
        return _NC_CACHE[key]
    (ws0, ws1, sel_mode, has_br) = key
    win_sizes = [list(ws0), list(ws1)]
    scheds = [_eye_sched(win_sizes[0]), _eye_sched(win_sizes[1])]
    nchunks = [len(scheds[0][0]), len(scheds[1][0])]
    n_win = SEGS // WIN

    nc = bacc.Bacc("TRN2", target_bir_lowering=False, debug=False)

    xp_d = [nc.dram_tensor(f"x{e}p", [128, nchunks[e] * 64], bf16,
                           kind="ExternalInput") for e in range(2)]
    wr2_d = [nc.dram_tensor(f"wr{e}", [128, R_OUT], bf16, kind="ExternalInput")
             for e in range(2)]
    wc_d = [nc.dram_tensor(f"wc{e}", [R_OUT, C_OUT], bf16, kind="ExternalInput")
            for e in range(2)]
    bc_d = [nc.dram_tensor(f"bc{e}", [C_OUT, 1], f32, kind="ExternalInput")
            for e in range(2)]
    if sel_mode == "dve":
        seg_d = [nc.dram_tensor(f"seg{e}", [128, nchunks[e]], f32,
                                kind="ExternalInput") for e in range(2)]
        iota_d = nc.dram_tensor("iota", [128, WIN], bf16, kind="ExternalInput")
        sel_d = None
        seldt = bf16
    else:
        seldt = bf16 if sel_mode == "bf16" else fp8e4
        sel_d = [nc.dram_tensor(f"sel{e}", [128, nchunks[e] * WIN], seldt,
                                kind="ExternalInput") for e in range(2)]
        seg_d = None
    if has_br:
        invr_d = [nc.dram_tensor(f"invr{e}", [1, nchunks[e] * 128], f32,
                                 kind="ExternalInput") for e in range(2)]
        br_d = [nc.dram_tensor(f"br{e}", [1, R_OUT], bf16, kind="ExternalInput")
                for e in range(2)]
    wb1lo_d = nc.dram_tensor("wb1lo", [128, BRAIN_H], bf16, kind="ExternalInput")
    wb1hi_d = nc.dram_tensor("wb1hi", [128, BRAIN_H], bf16, kind="ExternalInput")
    bb1a_d = nc.dram_tensor("bb1a", [128, 1], f32, kind="ExternalInput")
    bb1b_d = nc.dram_tensor("bb1b", [128, 1], f32, kind="ExternalInput")
    wb2lo_d = nc.dram_tensor("wb2lo", [128, BRAIN_OUT], bf16, kind="ExternalInput")
    wb2hi_d = nc.dram_tensor("wb2hi", [128, BRAIN_OUT], bf16, kind="ExternalInput")
    bb2_d = nc.dram_tensor("bb2", [BRAIN_OUT, 1], f32, kind="ExternalInput")
    outT_d = nc.dram_tensor("outT", [128, SEGS], f32, kind="ExternalOutput")

    with tile.TileContext(nc) as tc:
        with tc.tile_pool(name="consts", bufs=1) as cp:
            wr2_t = [cp.tile([128, R_OUT], bf16, tag=f"wr{e}", name=f"wr{e}t")
                     for e in range(2)]
            wc_t = [cp.tile([R_OUT, C_OUT], bf16, tag=f"wc{e}", name=f"wc{e}t")
                    for e in range(2)]
            bc_t = [cp.tile([C_OUT, 1], f32, tag=f"bc{e}", name=f"bc{e}t")
                    for e in range(2)]
            wb1lo_t = cp.tile([128, BRAIN_H], bf16, tag="wb1lo")
            wb1hi_t = cp.tile([128, BRAIN_H], bf16, tag="wb1hi")
            bb1a_t = cp.tile([128, 1], f32, tag="bb1a")
            bb1b_t = cp.tile([128, 1], f32, tag="bb1b")
            wb2lo_t = cp.tile([128, BRAIN_OUT], bf16, tag="wb2lo")
            wb2hi_t = cp.tile([128, BRAIN_OUT], bf16, tag="wb2hi")
            bb2_t = cp.tile([BRAIN_OUT, 1], f32, tag="bb2")
            for e in range(2):
                nc.sync.dma_start(out=wr2_t[e][:], in_=wr2_d[e][:])
                nc.sync.dma_start(out=wc_t[e][:], in_=wc_d[e][:])
                nc.sync.dma_start(out=bc_t[e][:], in_=bc_d[e][:])
            nc.sync.dma_start(out=wb1lo_t[:], in_=wb1lo_d[:])
            nc.sync.dma_start(out=wb1hi_t[:], in_=wb1hi_d[:])
            nc.sync.dma_start(out=bb1a_t[:], in_=bb1a_d[:])
            nc.sync.dma_start(out=bb1b_t[:], in_=bb1b_d[:])
            nc.sync.dma_start(out=wb2lo_t[:], in_=wb2lo_d[:])
            nc.sync.dma_start(out=wb2hi_t[:], in_=wb2hi_d[:])
            nc.sync.dma_start(out=bb2_t[:], in_=bb2_d[:])
            if sel_mode == "dve":
                iota_t = cp.tile([128, WIN], bf16, tag="iota")
                nc.sync.dma_start(out=iota_t[:], in_=iota_d[:])
                seg_t = [cp.tile([128, nchunks[e]], f32, tag=f"seg{e}",
                                 name=f"seg{e}t") for e in range(2)]
                for e in range(2):
                    nc.sync.dma_start(out=seg_t[e][:], in_=seg_d[e][:])
            if has_br:
                br_t = [cp.tile([1, R_OUT], bf16, tag=f"br{e}", name=f"br{e}t")
                        for e in range(2)]
                for e in range(2):
                    nc.sync.dma_start(out=br_t[e][:], in_=br_d[e][:])

            eT_t = [cp.tile([128, SEGS], bf16, tag=f"eT{e}", name=f"eT{e}t")
                    for e in range(2)]

            # ------------------------------------------------ main phase
            with (
                tc.tile_pool(name="xch", bufs=3) as xpool,
                tc.tile_pool(name="selp", bufs=4) as selp,
                tc.tile_pool(name="hs", bufs=4) as hpool,
                tc.tile_pool(name="fins", bufs=2) as fs,
                tc.tile_pool(name="invp", bufs=2) as invp,
                tc.tile_pool(name="bs", bufs=3) as bs,
                tc.tile_pool(name="hps", bufs=2, space="PSUM") as hpp,
                tc.tile_pool(name="winp", bufs=1, space="PSUM") as wpp,
                tc.tile_pool(name="wcp", bufs=1, space="PSUM") as wcp,
                tc.tile_pool(name="bph", bufs=1, space="PSUM") as bph,
                tc.tile_pool(name="bpy", bufs=1, space="PSUM") as bpy,
            ):
                def brain_tile(t):
                    r0 = t * 512
                    e0s = eT_t[0][:, r0:r0 + 512]
                    e1s = eT_t[1][:, r0:r0 + 512]
                    psh_a = bph.tile([128, 512], f32, tag="bph",
                                     name=f"pha{t}")
                    nc.tensor.matmul(out=psh_a[:], lhsT=wb1lo_t[:, 0:128],
                                     rhs=e0s, start=True, stop=False)
                    nc.tensor.matmul(out=psh_a[:], lhsT=wb1hi_t[:, 0:128],
                                     rhs=e1s, start=False, stop=True)
                    hTa = bs.tile([128, 512], bf16, tag="hTa", name=f"hTa{t}")
                    nc.scalar.activation(out=hTa[:], in_=psh_a[:], func=RELU,
                                         bias=bb1a_t[:, 0:1])
                    psh_b = bph.tile([128, 512], f32, tag="bph",
                                     name=f"phb{t}")
                    nc.tensor.matmul(out=psh_b[:], lhsT=wb1lo_t[:, 128:256],
                                     rhs=e0s, start=True, stop=False)
                    nc.tensor.matmul(out=psh_b[:], lhsT=wb1hi_t[:, 128:256],
                                     rhs=e1s, start=False, stop=True)
                    hTb = bs.tile([128, 512], bf16, tag="hTb", name=f"hTb{t}")
                    nc.scalar.activation(out=hTb[:], in_=psh_b[:], func=RELU,
                                         bias=bb1b_t[:, 0:1])
                    psy = bpy.tile([128, 512], f32, tag="bpy", name=f"py{t}")
                    nc.tensor.matmul(out=psy[:], lhsT=wb2lo_t[:], rhs=hTa[:],
                                     start=True, stop=False)
                    nc.tensor.matmul(out=psy[:], lhsT=wb2hi_t[:], rhs=hTb[:],
                                     start=False, stop=True)
                    ys = bs.tile([128, 512], f32, tag="ys", name=f"ys{t}")
                    nc.vector.tensor_scalar_add(ys[:], psy[:], bb2_t[:, 0:1])
                    nc.sync.dma_start(out=outT_d[:, r0:r0 + 512], in_=ys[:])

                relu_ct = 0
                for e in range(2):
                    woc, wfirst, wlast = scheds[e]
                    nch = nchunks[e]
                    xt = None
                    invt = None
                    selt = None
                    wacc = None
                    pending = None

                    def emit_mm2(c0, n, hsb, state):
                        nonlocal wacc, selt
                        for j in range(n):
                            c = c0 + j
                            w = woc[c]
                            g = w // WGRP
                            if sel_mode == "dve":
                                sel = selp.tile([128, WIN], bf16, tag="sel",
                                                name=f"sel{e}_{c}")
                                nc.vector.tensor_scalar(
                                    out=sel[:], in0=iota_t[:],
                                    scalar1=seg_t[e][:, c:c + 1],
                                    scalar2=None,
                                    op0=mybir.AluOpType.is_equal)
                                rhs = sel[:]
                            else:
                                if c % SELCH == 0:
                                    scnt = min(SELCH, nch - c)
                                    selt = selp.tile([128, SELCH * WIN], seldt,
                                                     tag="selt",
                                                     name=f"selt{e}_{c}")
                                    nc.sync.dma_start(
                                        out=selt[:, : scnt * WIN],
                                        in_=sel_d[e][:, c * WIN:
                                                     (c + scnt) * WIN])
                                off = (c % SELCH) * WIN
                                rhs = selt[:, off:off + WIN]
                            if c == wfirst[g * WGRP]:
                                wacc = wpp.tile([128, WGRP * WIN], f32,
                                                tag="wacc", name=f"wa{e}_{g}")
                            ws = (w % WGRP) * WIN
                            slot = (j >> 1) + (j & 1) * (HB // 2)
                            nc.tensor.matmul(
                                out=wacc[:, ws:ws + WIN],
                                lhsT=hsb[:, slot * 128:(slot + 1) * 128],
                                rhs=rhs,
                                start=(c == wfirst[w]),
                                stop=(c == wlast[w]),
                            )
                            if c == wlast[w] and w % WGRP == WGRP - 1:
                                meanT = fs.tile([128, WGRP * WIN], bf16,
                                                tag="meanT", name=f"mt{e}_{g}")
                                nc.vector.tensor_copy(meanT[:], wacc[:])
                                pse = wcp.tile([128, WGRP * WIN], f32,
                                               tag="pse", name=f"pse{e}_{g}")
                                nc.tensor.matmul(out=pse[:], lhsT=wc_t[e][:],
                                                 rhs=meanT[:],
                                                 start=True, stop=True)
                                nc.scalar.activation(
                                    out=eT_t[e][:, g * 512:(g + 1) * 512],
                                    in_=pse[:], func=RELU,
                                    bias=bc_t[e][:, 0:1])
                                if e == 1:
                                    brain_tile(g)

                    for c0 in range(0, nch, HB):
                        n = min(HB, nch - c0)
                        hps = hpp.tile([128, HB * 128], f32, tag="hps",
                                       name=f"hps{e}_{c0}")
                        hsb = hpool.tile([128, HB * 128], bf16, tag="hsb",
                                         name=f"hsb{e}_{c0}")
                        for t in range(n // 2):
                            pair = c0 // 2 + t
                            if pair % (XCOLS // 128) == 0:
                                pbase = pair * 128
                                pcsz = min(XCOLS, nch * 64 - pbase)
                                xt = xpool.tile([128, XCOLS], bf16, tag="xch",
                                                name=f"xch{e}_{pair}")
                                nc.sync.dma_start(
                                    out=xt[:, :pcsz],
                                    in_=xp_d[e][:, pbase:pbase + pcsz])
                                if has_br:
                                    ibase = pair * 256
                                    icsz = min(2 * XCOLS, nch * 128 - ibase)
                                    invt = invp.tile([1, 2 * XCOLS], f32,
                                                     tag="invr",
                                                     name=f"invr{e}_{pair}")
                                    nc.sync.dma_start(
                                        out=invt[:, :icsz],
                                        in_=invr_d[e][:, ibase:ibase + icsz])
                            col = (pair % (XCOLS // 128)) * 128
                            for half in range(2):
                                # row-tiled pair: A -> bank0 slot t,
                                # B -> bank1 slot HB//2+t (concurrent row
                                # tiles must write different PSUM banks)
                                slot = t + half * (HB // 2)
                                hs = slice(slot * 128, (slot + 1) * 128)
                                pb = half * 64
                                nc.tensor.matmul(
                                    out=hps[:, hs],
                                    lhsT=xt[pb:pb + 64, col:col + 128],
                                    rhs=wr2_t[e][pb:pb + 64, :],
                                    start=True, stop=not has_br,
                                )
                                if has_br:
                                    ic = (pair % (XCOLS // 128)) * 256 \
                                        + half * 128
                                    nc.tensor.matmul(
                                        out=hps[:, hs],
                                        lhsT=invt[0:1, ic:ic + 128],
                                        rhs=br_t[e][0:1, :],
                                        start=False, stop=True,
                                    )
                        if n == HB:
                            spans = [(0, HB * 128)]
                        else:
                            spans = [(0, (n // 2) * 128),
                                     ((HB // 2) * 128,
                                      (HB // 2) * 128 + (n // 2) * 128)]
                        for hh, hsz in spans:
                            if relu_ct % 2 < 1:
                                nc.scalar.activation(
                                    out=hsb[:, hh:hh + hsz],
                                    in_=hps[:, hh:hh + hsz], func=RELU)
                            else:
                                nc.vector.tensor_scalar_max(
                                    hsb[:, hh:hh + hsz],
                                    hps[:, hh:hh + hsz], 0.0)
                            relu_ct += 1
                        if pending is not None:
                            emit_mm2(*pending, None)
                        pending = (c0, n, hsb)
                    emit_mm2(*pending, None)
                    pending = None

    nc.compile()
    _NC_CACHE[key] = nc
    return nc


# ------------------------------------------------------------------ driver

def _prepare(inputs):
    x = [np.asarray(inputs["x0"], np.float32),
         np.asarray(inputs["x1"], np.float32)]
    idx = [np.asarray(inputs["idx0"]).astype(np.int64),
           np.asarray(inputs["idx1"]).astype(np.int64)]
    br = [np.asarray(inputs["br0"], np.float32),
          np.asarray(inputs["br1"], np.float32)]
    has_br = bool(np.any(br[0]) or np.any(br[1]))

    plans = [_plan_eye(idx[0]), _plan_eye(idx[1])]
    win_sizes = [plans[0][0], plans[1][0]]
    win_base = [np.cumsum([0] + ws) for ws in win_sizes]
    totals = [int(sum(ws)) for ws in win_sizes]
    n_win = SEGS // WIN

    invc = [
        (1.0 / np.maximum(
            np.bincount(idx[e], minlength=B_FULL), 1)).astype(np.float32)
        for e in range(2)
    ]

    shared = {}
    for e in range(2):
        wr = np.asarray(inputs[f"Wr{e}"], np.float32)
        shared[f"wr{e}"] = np.concatenate([wr, wr], axis=0).astype(BF16)
        shared[f"wc{e}"] = np.asarray(inputs[f"Wc{e}"], np.float32).astype(BF16)
        shared[f"bc{e}"] = np.asarray(
            inputs[f"bc{e}"], np.float32).reshape(-1, 1)
        if has_br:
            shared[f"br{e}"] = br[e].astype(BF16).reshape(1, -1)
    wb1 = np.asarray(inputs["Wb1"], np.float32)
    bb1 = np.asarray(inputs["bb1"], np.float32)
    wb2 = np.asarray(inputs["Wb2"], np.float32)
    bb2 = np.asarray(inputs["bb2"], np.float32)
    shared["wb1lo"] = wb1[0:128].astype(BF16)
    shared["wb1hi"] = wb1[128:256].astype(BF16)
    shared["bb1a"] = bb1[0:128].reshape(-1, 1)
    shared["bb1b"] = bb1[128:256].reshape(-1, 1)
    shared["wb2lo"] = wb2[0:128].astype(BF16)
    shared["wb2hi"] = wb2[128:256].astype(BF16)
    shared["bb2"] = bb2.reshape(-1, 1)
    if SEL_MODE == "dve":
        shared["iota"] = np.broadcast_to(
            np.arange(WIN, dtype=np.float32), (128, WIN)).astype(BF16)

    in_maps = []
    for c in range(CORES):
        m = dict(shared)
        for e in range(2):
            nodes, srel = plans[e][1][c]
            total = totals[e]
            nchunks = total // 128
            # stream position: window-aligned with per-window padding
            wid = srel // WIN
            wstart = np.searchsorted(wid, np.arange(n_win))
            pos = np.empty(len(nodes), np.int64)
            for w in range(n_win):
                lo = wstart[w]
                hi = wstart[w + 1] if w + 1 < n_win else len(nodes)
                pos[lo:hi] = win_base[e][w] + np.arange(hi - lo)
            arr = np.zeros((total, IN_NF), np.float32)
            arr[pos] = x[e][nodes] * invc[e][idx[e][nodes]][:, None]
            a3 = arr.reshape(nchunks, 128, IN_NF).astype(BF16)
            xp = np.concatenate([a3[0::2], a3[1::2]], axis=2)  # [P, 128, 128]
            m[f"x{e}p"] = np.ascontiguousarray(
                xp.transpose(2, 0, 1).reshape(128, total // 2))
            segv = np.full(total, -1.0, np.float32)
            segv[pos] = (srel % WIN).astype(np.float32)
            if SEL_MODE == "dve":
                m[f"seg{e}"] = np.ascontiguousarray(
                    segv.reshape(nchunks, 128).T)
            else:
                sdt = BF16 if SEL_MODE == "bf16" else FP8
                sel = (segv[:, None] ==
                       np.arange(WIN, dtype=np.float32)[None, :])
                m[f"sel{e}"] = np.ascontiguousarray(
                    sel.reshape(nchunks, 128, WIN).transpose(1, 0, 2)
                    .reshape(128, nchunks * WIN)).astype(sdt)
            if has_br:
                iv = np.zeros(total, np.float32)
                iv[pos] = invc[e][idx[e][nodes]]
                m[f"invr{e}"] = iv.reshape(1, total)
        in_maps.append(m)
    key = (tuple(win_sizes[0]), tuple(win_sizes[1]), SEL_MODE, has_br)
    return key, in_maps


def _axon_reset():
    try:
        import ctypes

        lib = ctypes.CDLL("/opt/axon/libaxon_pjrt.so")
        lib.axon_reset.restype = ctypes.c_int
        lib.axon_reset()
    except Exception:
        pass


def _run(inputs, trace=False, trace_kwargs=None):
    key, in_maps = _prepare(inputs)
    nc = _build_nc(key)
    try:
        res = run_bass_kernel_spmd(nc, in_maps, list(range(CORES)),
                                   trace=trace, **(trace_kwargs or {}))
    except Exception as e:
        if "UNRECOVERABLE" not in str(e) and "UNAVAILABLE" not in str(e):
            raise
        _axon_reset()
        res = run_bass_kernel_spmd(nc, in_maps, list(range(CORES)),
                                   trace=trace, **(trace_kwargs or {}))
    out = np.concatenate([res.results[c]["outT"].T for c in range(CORES)],
                         axis=0)
    return out.astype(np.float32), res


def kernel(**inputs):
    return _run(inputs)[0]
